# revision 1
# baseline (speedup 1.0000x reference)
"""Trainium2 Bass kernel for CRKT layer (decay-reweighted causal attention).

Math per batch b (one NeuronCore per batch element, 8 cores):
  q = query @ Wq.T + bq ; k = key_in @ Wq.T + bq ; v = value @ Wv.T + bv
  s = q k^T / sqrt(dk)  (per head, causal)
  a1 = softmax(s)                       [unnormalized expS + row-sum Z]
  suffix[i,j] = sum_{j'>j} a1[i,j'] = (Z - cumsum(expS))/Z
  dist = (i-j) * suffix ;  te = exp(-|decay_h| * dist)
  a2 = softmax(s * te) ;  out = a2 @ v ;  y = LN(out @ Wo.T + bo) * gamma + beta

Layout strategy (per core):
  - inputs transposed on-chip via PE (identity matmul), projections in fp32r
  - per (head, i-tile) pipeline in [i, j] layout: exp1 (ACT, scale=1/8)
    -> causal mask (gpsimd affine_select) -> cumsum (DVE scan) -> dist
    (DVE scalar_tensor_tensor with iota |i-j| tiles) -> te (ACT exp with
    per-partition scale |decay|/Z) -> s2 = s*te (DVE) -> diag mask to -1e30
    -> exp2 (ACT, bf16 out, accum_out = Z2)
  - PV needs alpha^T: alphaT = P2^T @ diag(1/Z2) as plain bf16 matmuls
    (normalization folded into the transpose) -> strips [j, i]
  - outT[dk, i] = V^T-free matmul: lhsT = V block, rhs = alphaT strip
  - y = (OT^T @ WoT) + bo -> LayerNorm -> gamma/beta -> DMA out
"""

import sys

for _p in ("/opt/trn_rl_repo",):
    if _p not in sys.path:
        sys.path.insert(0, _p)

import numpy as np

import concourse.bass as bass
import concourse.mybir as mybir
import concourse.tile as tile
from concourse import bacc, bass_utils
from concourse.masks import make_identity

F32 = mybir.dt.float32
F32R = mybir.dt.float32r
BF16 = mybir.dt.bfloat16
AL = mybir.AluOpType
AF = mybir.ActivationFunctionType

S, DIM, H, DK = 1024, 512, 8, 64
T = S // 128        # 8 i-tiles
NB = S // 128       # 8 j-blocks
NEGBIG = -1e30

_CACHE = {}


def _chunks(total, step):
    return [(a, min(a + step, total)) for a in range(0, total, step)]


def build():
    nc = bacc.Bacc("TRN2", target_bir_lowering=False, debug=False, num_devices=8)

    d_query = nc.dram_tensor("query", [S, DIM], F32, kind="ExternalInput")
    d_key = nc.dram_tensor("key_in", [S, DIM], F32, kind="ExternalInput")
    d_value = nc.dram_tensor("value", [S, DIM], F32, kind="ExternalInput")
    d_wq = nc.dram_tensor("Wq", [DIM, DIM], F32, kind="ExternalInput")
    d_wv = nc.dram_tensor("Wv", [DIM, DIM], F32, kind="ExternalInput")
    d_wo = nc.dram_tensor("Wo", [DIM, DIM], F32, kind="ExternalInput")
    d_bq = nc.dram_tensor("bq", [1, DIM], F32, kind="ExternalInput")
    d_bv = nc.dram_tensor("bv", [1, DIM], F32, kind="ExternalInput")
    d_bo = nc.dram_tensor("bo", [1, DIM], F32, kind="ExternalInput")
    d_dec = nc.dram_tensor("decay", [1, H], F32, kind="ExternalInput")
    d_gam = nc.dram_tensor("gamma", [1, DIM], F32, kind="ExternalInput")
    d_bet = nc.dram_tensor("beta", [1, DIM], F32, kind="ExternalInput")
    d_out = nc.dram_tensor("out", [S, DIM], F32, kind="ExternalOutput")

    with tile.TileContext(nc) as tc:
        _body(nc, tc, d_query, d_key, d_value, d_wq, d_wv, d_wo,
              d_bq, d_bv, d_bo, d_dec, d_gam, d_bet, d_out)

    nc.compile()
    return nc


def _body(nc, tc, d_query, d_key, d_value, d_wq, d_wv, d_wo,
          d_bq, d_bv, d_bo, d_dec, d_gam, d_bet, d_out):
    import contextlib
    ctx = contextlib.ExitStack()
    with ctx:
        const = ctx.enter_context(tc.tile_pool(name="const", bufs=1))
        persist = ctx.enter_context(tc.tile_pool(name="persist", bufs=1))

        # ---- constants ----
        ident = const.tile([128, 128], F32)
        make_identity(nc, ident[:])
        identb = const.tile([128, 128], BF16)
        make_identity(nc, identb[:])
        identr = const.tile([128, 128], F32R)
        nc.vector.tensor_copy(identr[:], ident[:])
        cmask = const.tile([128, 128], F32)
        nc.gpsimd.memset(cmask[:], 0.0)
        nc.gpsimd.affine_select(
            out=cmask[:], in_=cmask[:], compare_op=AL.is_ge, fill=NEGBIG,
            base=0, channel_multiplier=1, pattern=[[-1, 128]])
        cmaskr = const.tile([128, 128], F32R)
        nc.vector.tensor_copy(cmaskr[:], cmask[:])
        eps_t = const.tile([128, 1], F32)
        nc.vector.memset(eps_t[:], 1e-5)

        bq_sb = const.tile([128, 4], F32)   # col m = bq[128m:128(m+1)]
        nc.sync.dma_start(bq_sb[:], d_bq[0, :].rearrange("(m p) -> p m", p=128))
        bv_bc = const.tile([128, DIM], F32)
        nc.sync.dma_start(bv_bc[:], d_bv.ap().to_broadcast((128, DIM)))
        bo_bc = const.tile([128, DIM], F32)
        nc.sync.dma_start(bo_bc[:], d_bo.ap().to_broadcast((128, DIM)))
        gam_bc = const.tile([128, DIM], F32)
        nc.sync.dma_start(gam_bc[:], d_gam.ap().to_broadcast((128, DIM)))
        bet_bc = const.tile([128, DIM], F32)
        nc.sync.dma_start(bet_bc[:], d_bet.ap().to_broadcast((128, DIM)))

        lam = const.tile([128, H], F32)     # |decay_h| broadcast down partitions
        nc.sync.dma_start(lam[:], d_dec.ap().to_broadcast((128, H)))
        nc.scalar.activation(lam[:], lam[:], AF.Abs)

        # D tiles: d_t[p, j] = 128 t + p - j   (= i - j), j in [0, L_t)
        d_tiles = []
        for t in range(T):
            L = 128 * (t + 1)
            dt_ = const.tile([128, L], BF16, tag=f"dti_{t}", name=f"dti_{t}")
            d_tiles.append(dt_)

        # ---- load & transpose weights and inputs; projections ----
        qt = [persist.tile([128, S], F32R, tag=f"qt{g}", name=f"qt{g}") for g in range(4)]
        kt = [persist.tile([128, S], F32R, tag=f"kt{g}", name=f"kt{g}") for g in range(4)]
        v_sb = [persist.tile([128, DIM], BF16, tag=f"v{t}", name=f"v{t}") for t in range(T)]
        ot_sb = [persist.tile([128, S], BF16, tag=f"ot{g}", name=f"ot{g}") for g in range(4)]
        wot = [persist.tile([128, DIM], BF16, tag=f"wot{g}", name=f"wot{g}") for g in range(4)]

        with tc.tile_pool(name="stage", bufs=2) as stage, \
             tc.tile_pool(name="pstage", bufs=4, space="PSUM") as pstage:

            for t in range(T):
                L = 128 * (t + 1)
                dsc = stage.tile([128, S], F32, tag="dscratch", name=f"dsc{t}")
                nc.gpsimd.iota(dsc[:, :L], pattern=[[-1, L]], base=128 * t,
                               channel_multiplier=1,
                               allow_small_or_imprecise_dtypes=True)
                nc.gpsimd.tensor_copy(d_tiles[t][:], dsc[:, :L])

            # transpose weights -> WT (fp32r), keep WoT persistent
            wts = {}
            for name, dram in (("wq", d_wq), ("wv", d_wv), ("wo", d_wo)):
                w_rows = []
                for r in range(4):
                    wr = stage.tile([128, DIM], F32, tag=f"wld{r}")
                    nc.sync.dma_start(wr[:], dram[128 * r:128 * (r + 1), :])
                    w_rows.append(wr)
                cols = []
                for c in range(4):
                    pt = pstage.tile([128, DIM], F32, tag="ptr")
                    for r in range(4):
                        nc.tensor.transpose(pt[:, 128 * r:128 * (r + 1)],
                                            w_rows[r][:, 128 * c:128 * (c + 1)],
                                            ident[:])
                    if name == "wo":
                        dst = wot[c]
                    else:
                        dst = persist.tile([128, DIM], F32R, tag=f"{name}t{c}", name=f"{name}t{c}")
                    nc.vector.tensor_copy(dst[:], pt[:])
                    cols.append(dst)
                wts[name] = cols

            # transpose inputs -> XT (fp32r) then project
            for name, dram in (("q", d_query), ("k", d_key), ("v", d_value)):
                x_rows = []
                for t in range(T):
                    xr = stage.tile([128, DIM], F32, tag=f"xld{t % 4}")
                    nc.sync.dma_start(xr[:], dram[128 * t:128 * (t + 1), :])
                    x_rows.append(xr)
                xt = []
                for d in range(4):
                    xtd = stage.tile([128, S], F32R, tag=f"xt{d}")
                    for half in range(2):
                        pt = pstage.tile([128, DIM], F32, tag="ptr")
                        for idx in range(4):
                            t = 4 * half + idx
                            nc.tensor.transpose(
                                pt[:, 128 * idx:128 * (idx + 1)],
                                x_rows[t][:, 128 * d:128 * (d + 1)], ident[:])
                        nc.vector.tensor_copy(
                            xtd[:, DIM * half:DIM * (half + 1)], pt[:])
                    xt.append(xtd)

                if name in ("q", "k"):
                    dst_tiles = qt if name == "q" else kt
                    for m in range(4):
                        for half in range(2):
                            pt = pstage.tile([128, DIM], F32, tag="pproj")
                            for kk in range(4):
                                nc.tensor.matmul(
                                    pt[:],
                                    wts["wq"][kk][:, 128 * m:128 * (m + 1)],
                                    xt[kk][:, DIM * half:DIM * (half + 1)],
                                    start=(kk == 0), stop=(kk == 3))
                            nc.scalar.activation(
                                dst_tiles[m][:, DIM * half:DIM * (half + 1)],
                                pt[:], AF.Identity,
                                bias=bq_sb[:, m:m + 1])
                else:
                    for t in range(T):
                        pt = pstage.tile([128, DIM], F32, tag="pproj")
                        for kk in range(4):
                            nc.tensor.matmul(
                                pt[:], xt[kk][:, 128 * t:128 * (t + 1)],
                                wts["wv"][kk][:],
                                start=(kk == 0), stop=(kk == 3))
                        nc.vector.tensor_add(v_sb[t][:], pt[:], bv_bc[:])

        # ---- attention per head ----
        with tc.tile_pool(name="pss", bufs=3, space="PSUM") as pss, \
             tc.tile_pool(name="psot", bufs=1, space="PSUM") as psot, \
             tc.tile_pool(name="sbA", bufs=4) as sbA, \
             tc.tile_pool(name="sbB", bufs=4) as sbB, \
             tc.tile_pool(name="sbS", bufs=8) as sbS, \
             tc.tile_pool(name="sbP", bufs=2) as sbP, \
                 tc.tile_pool(name="sbG", bufs=1) as sbG:

            pair_pot = {}
            p2s = {}
            tes = {}
            strips = {}

            def phase_a(h, t):
                g, off, par = h // 2, 64 * (h % 2), h % 2
                L = 128 * (t + 1)
                ps_s = pss.tile([128, S], F32, tag="s", name=f"ps_{h}_{t}")
                for (j0, j1) in _chunks(L, 512):
                    nc.tensor.matmul(
                        ps_s[:, j0:j1],
                        qt[g][off:off + 64, 128 * t:128 * (t + 1)],
                        kt[g][off:off + 64, j0:j1],
                        start=True, stop=(j1 < L))
                nc.tensor.matmul(ps_s[:, 128 * t:L], identr[:], cmaskr[:],
                                 start=False, stop=True)

                expS = sbA.tile([128, S], F32, tag="expS", name=f"e_{h}_{t}")
                nc.scalar.activation(expS[:, :L], ps_s[:, :L], AF.Exp,
                                     scale=0.125)
                scan = sbB.tile([128, S], F32, tag="scan", name=f"sc_{h}_{t}")
                nc.vector.tensor_tensor_scan(
                    scan[:, :L], expS[:, :L], expS[:, :L], 0.0,
                    op0=AL.add, op1=AL.bypass)
                sc1 = sbS.tile([128, 1], F32, tag="sc1", name=f"s1_{h}_{t}")
                nc.vector.reciprocal(sc1[:], scan[:, L - 1:L])
                nc.gpsimd.tensor_scalar_mul(sc1[:], in0=sc1[:],
                                            scalar1=lam[:, h:h + 1])
                z1 = scan[:, L - 1:L]
                if t < 4:
                    nc.gpsimd.tensor_scalar_sub(scan[:, :L], in0=scan[:, :L],
                                                scalar1=z1)
                    nc.gpsimd.tensor_mul(scan[:, :L], scan[:, :L],
                                         d_tiles[t][:, :L])
                else:
                    nc.vector.scalar_tensor_tensor(
                        scan[:, :L], in0=scan[:, :L], scalar=z1,
                        in1=d_tiles[t][:, :L], op0=AL.subtract, op1=AL.mult)
                te = sbP.tile([128, L], BF16, tag=f"te_{t}_{par}",
                              name=f"te_{h}_{t}", bufs=1)
                nc.scalar.activation(te[:, :L], scan[:, :L], AF.Exp,
                                     scale=sc1[:])
                tes[(h, t)] = te

            def phase_b(h, t):
                g, off, par = h // 2, 64 * (h % 2), h % 2
                L = 128 * (t + 1)
                if t == 0:
                    for b in range(NB):
                        strips[(h, b)] = sbP.tile(
                            [128, S], BF16, tag=f"st_{b}_{par}",
                            name=f"st_{h}_{b}", bufs=1)
                te = tes[(h, t)]
                ps_b = pss.tile([128, S], F32, tag="s", name=f"pb_{h}_{t}")
                for (j0, j1) in _chunks(L, 512):
                    nc.tensor.matmul(
                        ps_b[:, j0:j1],
                        qt[g][off:off + 64, 128 * t:128 * (t + 1)],
                        kt[g][off:off + 64, j0:j1],
                        start=True, stop=(j1 < L))
                nc.tensor.matmul(ps_b[:, 128 * t:L], identr[:], cmaskr[:],
                                 start=False, stop=True)
                s2 = sbB.tile([128, S], F32, tag="scan", name=f"s2_{h}_{t}")
                nc.vector.tensor_mul(s2[:, :L], ps_b[:, :L], te[:, :L])
                p2 = sbP.tile([128, L], BF16, tag=f"p2_{t}_{par}",
                              name=f"p2_{h}_{t}", bufs=1)
                z2 = sbS.tile([128, 1], F32, tag="z2", name=f"z2_{h}_{t}")
                nc.scalar.activation(p2[:, :L], s2[:, :L], AF.Exp,
                                     scale=0.125, accum_out=z2[:])
                rz2 = sbS.tile([128, 1], F32, tag="rz2", name=f"r2_{h}_{t}")
                nc.vector.reciprocal(rz2[:], z2[:])
                nc.vector.tensor_scalar_mul(p2[:, :L], in0=p2[:, :L],
                                            scalar1=rz2[:])
                p2s[(h, t)] = p2
                for b in range(t + 1):
                    nc.sync.dma_start_transpose(
                        strips[(h, b)][:, 128 * t:128 * (t + 1)],
                        p2[:, 128 * b:128 * (b + 1)])

            def ot_chunk(h, c):
                # outT[dk, i0:i0+512]; strips cols for chunk c are complete
                # once tile t=4c+3 has run phase_b. Head pair shares psum.
                g, off = h // 2, 64 * (h % 2)
                i0 = 512 * c
                if h % 2 == 0:
                    pair_pot[c] = psot.tile([128, DIM], F32, tag=f"ot{c}",
                                            name=f"pot{c}_{h}")
                pot = pair_pot[c]
                bs = [b for b in range(NB) if 128 * b < i0 + 512]
                for b in bs:
                    a0 = max(0, 128 * b - i0)
                    nc.tensor.matmul(
                        pot[off:off + 64, a0:512],
                        v_sb[b][:, 64 * h:64 * h + 64],
                        strips[(h, b)][:, i0 + a0:i0 + 512],
                        start=(b == 0), stop=(b == bs[-1]),
                        tile_position=(0, off))
                if h % 2 == 1:
                    nc.vector.tensor_copy(ot_sb[g][:, i0:i0 + 512], pot[:])

            for hp in range(4):
                h0, h1 = 2 * hp, 2 * hp + 1
                for t in range(T):
                    phase_a(h0, t)
                    phase_a(h1, t)
                    phase_b(h0, t)
                    phase_b(h1, t)
                ot_chunk(h0, 0)
                ot_chunk(h1, 0)
                ot_chunk(h0, 1)
                ot_chunk(h1, 1)

            # ---- output projection + LayerNorm ----
            for t in range(T):
                psy = pss.tile([128, DIM], F32, tag="s", name="psy")
                for g in range(4):
                    nc.tensor.matmul(psy[:],
                                     ot_sb[g][:, 128 * t:128 * (t + 1)],
                                     wot[g][:], start=(g == 0), stop=(g == 3))
                y1 = sbA.tile([128, DIM], F32, tag="y1")
                nc.vector.tensor_add(y1[:], psy[:], bo_bc[:])
                stats = sbS.tile([128, 6], F32, tag="bst")
                nc.vector.bn_stats(out=stats[:], in_=y1[:])
                mv = sbS.tile([128, 2], F32, tag="bmv")
                nc.vector.bn_aggr(out=mv[:], in_=stats[:])
                rstd = sbS.tile([128, 1], F32, tag="rstd")
                nc.scalar.activation(rstd[:], mv[:, 1:2], AF.Sqrt,
                                     bias=eps_t[:])
                nc.vector.reciprocal(rstd[:], rstd[:])
                y2 = sbB.tile([128, DIM], F32, tag="y2")
                nc.vector.tensor_scalar(out=y2[:], in0=y1[:],
                                        scalar1=mv[:, 0:1], scalar2=rstd[:],
                                        op0=AL.subtract, op1=AL.mult)
                nc.gpsimd.tensor_mul(y2[:], y2[:], gam_bc[:])
                nc.gpsimd.tensor_add(y2[:], y2[:], bet_bc[:])
                nc.sync.dma_start(d_out[128 * t:128 * (t + 1), :], y2[:])


def kernel(**inputs):
    query = np.asarray(inputs["query"], np.float32)
    key_in = np.asarray(inputs["key_in"], np.float32)
    value = np.asarray(inputs["value"], np.float32)
    B = query.shape[0]
    assert query.shape == (B, S, DIM)

    if "nc" not in _CACHE:
        _CACHE["nc"] = build()
    nc = _CACHE["nc"]

    base = {
        "Wq": np.asarray(inputs["Wq"], np.float32),
        "Wv": np.asarray(inputs["Wv"], np.float32),
        "Wo": np.asarray(inputs["Wo"], np.float32),
        "bq": np.asarray(inputs["bq"], np.float32).reshape(1, DIM),
        "bv": np.asarray(inputs["bv"], np.float32).reshape(1, DIM),
        "bo": np.asarray(inputs["bo"], np.float32).reshape(1, DIM),
        "decay": np.asarray(inputs["decay"], np.float32).reshape(1, H),
        "gamma": np.asarray(inputs["gamma"], np.float32).reshape(1, DIM),
        "beta": np.asarray(inputs["beta"], np.float32).reshape(1, DIM),
    }
    in_maps = []
    for c in range(8):
        b = min(c, B - 1)
        m = dict(base)
        m["query"] = np.ascontiguousarray(query[b])
        m["key_in"] = np.ascontiguousarray(key_in[b])
        m["value"] = np.ascontiguousarray(value[b])
        in_maps.append(m)

    res = bass_utils.run_bass_kernel_spmd(nc, in_maps, core_ids=list(range(8)))
    out = np.stack([res.results[c]["out"] for c in range(B)], 0)
    return out.astype(np.float32)

